# revision 18
# baseline (speedup 1.0000x reference)
"""Trainium2 Bass kernel for the attention-LSTM image-captioning decoder
(Show-Attend-Tell style). Full inputs in, full outputs out.

Sharding: data-parallel over batch across 8 NeuronCores (32 rows each),
weights replicated; the sequential 25-step time loop runs locally per core.

Per-core device algorithm (P=128, b=32, R=196, D=E=512, V=10000):
  index order i = r*32 + b; partition p = i%128 = (r%4)*32 + b; j = i//128.
  - W_s = img @ W_w.T + W_b once into SBUF (bf16, d-on-partitions, free=i)
  - per step: u = U h + U_b (PE); att = tanh(W_s + u) (DVE bcast-add + one
    big ACT tanh per chunk); e = v . att on PE with att as the stationary
    operand (M=128 i-chunk, N=1) accumulating the four d-chunks into psum
    column j -> e lands already "transposed" as [p=(r%4)*32+b, j=r//4].
    Softmax: exp with fused accum (row partial sums), 0/1-selection matmuls
    for the cross-partition group sums, reciprocal, scale. alpha -> bf16,
    zero-padded to 64 cols, one DVE 32x32 block-transpose gives alpha^T in
    exactly the (r%4)*32 + r//4 partition order that the host-prepped f
    tiles use; ctx = sum_r alpha f on PE with f stationary per (b, e-chunk).
    LSTM gates with W^T stationary; sigmoid = 0.5*tanh(x/2)+0.5 keeps all
    transcendentals in the exp/tanh activation-table set (no table reloads).
  - h archived per step; output projection done as [800,512]@[512,10000]
    with H stationary and out_w^T streamed from DRAM; out_b folded in via a
    K=1 ones-row matmul broadcast and a fused add on the PSUM->SBUF copy.
"""

import math
import numpy as np
import ml_dtypes
from contextlib import ExitStack

import concourse.bass as bass
import concourse.mybir as mybir
import concourse.tile as tile
from concourse import bacc
from concourse.bass import ts, ds
from concourse.masks import make_identity

P = 128
B, R, E, D, V, T = 256, 196, 512, 512, 10000, 26
NCORES = 8
NB = B // NCORES          # 32 batch rows per core
TT = T - 1                # 25 time steps
BR = NB * R               # 6272
NJ = BR // P              # 49
RPC = 20                  # r per attention chunk (20*32 = 5 i-subchunks)
BT = NB * TT              # 800
BTP = 896                 # padded to 7*128
F32 = mybir.dt.float32
BF16 = mybir.dt.bfloat16
I32 = mybir.dt.int32
AF = mybir.ActivationFunctionType
OP = mybir.AluOpType
bf16 = ml_dtypes.bfloat16


def build_nc(tsteps=TT):
    nc = bacc.Bacc("TRN2", target_bir_lowering=False, debug=False)
    dp = nc.declare_dram_parameter
    # per-core tensors
    img_eT = dp("img_eT", [P, 4, BR], BF16, isOutput=False)   # [e_in, e_out, i]
    f0 = dp("f0", [P, NB, E], BF16, isOutput=False)           # r' = 0..127 as (r%4)*32+r//4
    f1 = dp("f1", [P, NB, E], BF16, isOutput=False)           # r' = 128..255, zero padded
    cap = dp("cap", [P, 7], I32, isOutput=False)              # emb row idx, bt-major
    # replicated weights (pre-transposed/cast on host)
    WwT = dp("WwT", [P, 4, D], BF16, isOutput=False)
    UwT = dp("UwT", [P, 4, D], BF16, isOutput=False)
    fbwT = dp("fbwT", [P, 4, E], BF16, isOutput=False)
    ihwT = dp("ihwT", [P, 4, D], BF16, isOutput=False)        # pre-scaled by 1/R
    icwT = dp("icwT", [P, 4, D], BF16, isOutput=False)
    WihT = dp("WihT", [P, 8, 4 * D], BF16, isOutput=False)
    WhhT = dp("WhhT", [P, 4, 4 * D], BF16, isOutput=False)
    owT = dp("owT", [P, 4, V], BF16, isOutput=False)
    owb = dp("owb", [1, V], BF16, isOutput=False)             # out_b row
    embt = dp("embt", [V, D], BF16, isOutput=False)
    v4 = dp("v4", [P, 4], BF16, isOutput=False)
    selbp = dp("selbp", [P, NB], F32, isOutput=False)
    selpb = dp("selpb", [NB, P], F32, isOutput=False)
    Ub4 = dp("Ub4", [P, 4], F32, isOutput=False)
    Wb4 = dp("Wb4", [P, 4], F32, isOutput=False)
    fbb4h = dp("fbb4h", [P, 4], F32, isOutput=False)          # fb_b / 2
    ihb4 = dp("ihb4", [P, 4], F32, isOutput=False)
    icb4 = dp("icb4", [P, 4], F32, isOutput=False)
    bih16 = dp("bih16", [P, 16], F32, isOutput=False)         # b_ih + b_hh
    preds = dp("preds", [NB, tsteps, V], F32, isOutput=True)
    alphas = dp("alphas", [NB, tsteps, R], F32, isOutput=True)

    with tile.TileContext(nc) as tc:
        with ExitStack() as ctx:
            build_body(ctx, tc, tsteps, img_eT, f0, f1, cap, WwT, UwT, fbwT,
                       ihwT, icwT, WihT, WhhT, owT, owb, embt, v4, selbp,
                       selpb, Ub4, Wb4, fbb4h, ihb4, icb4, bih16, preds,
                       alphas)
    nc.compile()
    return nc


def build_body(ctx, tc, tsteps, img_eT, f0, f1, cap, WwT, UwT, fbwT, ihwT,
               icwT, WihT, WhhT, owT, owb, embt, v4, selbp, selpb, Ub4, Wb4,
               fbb4h, ihb4, icb4, bih16, preds, alphas):
    nc = tc.nc
    bt = NB * tsteps
    ngr = math.ceil(bt / P)
    btp = ngr * P
    const = ctx.enter_context(tc.tile_pool(name="const", bufs=1))

    # ---- resident SBUF tiles ----
    fr0 = const.tile([P, NB, E], BF16, tag="fr0")
    fr1 = const.tile([P, NB, E], BF16, tag="fr1")
    nc.sync.dma_start(fr0[:], f0[:])
    nc.sync.dma_start(fr1[:], f1[:])
    wih = const.tile([P, 8, 4 * D], BF16, tag="wih")
    whh = const.tile([P, 4, 4 * D], BF16, tag="whh")
    uw = const.tile([P, 4, D], BF16, tag="uw")
    fbw = const.tile([P, 4, E], BF16, tag="fbw")
    nc.sync.dma_start(wih[:], WihT[:])
    nc.sync.dma_start(whh[:], WhhT[:])
    nc.sync.dma_start(uw[:], UwT[:])
    nc.sync.dma_start(fbw[:], fbwT[:])
    v4t = const.tile([P, 4], BF16, tag="v4t")
    nc.sync.dma_start(v4t[:], v4[:])
    sbp = const.tile([P, NB], F32, tag="sbp")
    spb = const.tile([NB, P], F32, tag="spb")
    nc.sync.dma_start(sbp[:], selbp[:])
    nc.sync.dma_start(spb[:], selpb[:])
    ub4 = const.tile([P, 4], F32, tag="ub4")
    wb4 = const.tile([P, 4], F32, tag="wb4")
    fbb = const.tile([P, 4], F32, tag="fbb")
    ihb = const.tile([P, 4], F32, tag="ihb")
    icb = const.tile([P, 4], F32, tag="icb")
    b16 = const.tile([P, 16], F32, tag="b16")
    nc.sync.dma_start(ub4[:], Ub4[:])
    nc.sync.dma_start(wb4[:], Wb4[:])
    nc.sync.dma_start(fbb[:], fbb4h[:])
    nc.sync.dma_start(ihb[:], ihb4[:])
    nc.sync.dma_start(icb[:], icb4[:])
    nc.sync.dma_start(b16[:], bih16[:])
    capt = const.tile([P, 7], I32, tag="capt")
    nc.sync.dma_start(capt[:], cap[:])

    ws = const.tile([P, 4, BR], BF16, tag="ws")        # W_s, free order i
    embT = const.tile([P, 4, btp], BF16, tag="embT")   # teacher-forced emb^T
    H = const.tile([P, 4, btp], BF16, tag="H")         # h archive (post-step)
    h0 = const.tile([P, 4, NB], BF16, tag="h0")
    c = const.tile([P, 4, NB], F32, tag="c")
    apad = const.tile([P, 64], BF16, tag="apad")       # alpha padded to 64 cols
    nc.gpsimd.memset(apad[:], 0.0)
    ones1 = const.tile([P, 1], BF16, tag="ones1")
    nc.gpsimd.memset(ones1[:], 1.0)
    onesr = const.tile([1, P], BF16, tag="onesr")
    nc.gpsimd.memset(onesr[:], 1.0)
    alf_all = const.tile([P, tsteps, NJ], F32, tag="alf_all")

    # ---- prologue ----
    with tc.tile_pool(name="pro", bufs=1) as pro, \
         tc.tile_pool(name="pro2", bufs=2) as pro2, \
         tc.tile_pool(name="prop", bufs=1, space="PSUM") as prop, \
         tc.tile_pool(name="prow", bufs=2, space="PSUM") as prow:
        ident = pro.tile([P, P], BF16, tag="ident")
        make_identity(nc, ident[:])
        # emb gather + transpose to embT
        for g in range(ngr):
            gt = pro2.tile([P, D], BF16, tag="gather")
            nc.gpsimd.indirect_dma_start(
                out=gt[:], out_offset=None, in_=embt[:],
                in_offset=bass.IndirectOffsetOnAxis(ap=capt[:, g:g + 1], axis=0))
            for dc in range(4):
                pst = prop.tile([P, P], BF16, tag="tpsum")
                nc.tensor.transpose(pst[:], gt[:, ts(dc, P)], ident[:])
                nc.vector.tensor_copy(out=embT[:, dc, ts(g, P)], in_=pst[:])

        # batch-sum of img over r via PE (f stationary, ones rhs);
        # the 1/R mean scale is folded into ihwT/icwT on the host.
        psa = prop.tile([P, 4, NB], F32, tag="avgp")
        for b in range(NB):
            for ec in range(4):
                nc.tensor.matmul(psa[:, ec, b:b + 1], lhsT=fr0[:, b, ts(ec, P)],
                                 rhs=ones1[:], start=True, stop=False)
                nc.tensor.matmul(psa[:, ec, b:b + 1], lhsT=fr1[:, b, ts(ec, P)],
                                 rhs=ones1[:], start=False, stop=True)
        avg = pro.tile([P, 4, NB], BF16, tag="avg")
        nc.vector.tensor_copy(out=avg[:], in_=psa[:])

        # h0 = tanh(ih_w @ avg/R + ih_b), c0 likewise
        iw = pro.tile([P, 4, D], BF16, tag="iw")
        nc.sync.dma_start(iw[:], ihwT[:])
        ps0 = prop.tile([P, 4, NB], F32, tag="h0p")
        for mo in range(4):
            for ko in range(4):
                nc.tensor.matmul(ps0[:, mo, :], lhsT=iw[:, ko, ts(mo, P)],
                                 rhs=avg[:, ko, :], start=ko == 0, stop=ko == 3)
        for mo in range(4):
            nc.scalar.activation(h0[:, mo, :], ps0[:, mo, :], AF.Tanh,
                                 bias=ihb[:, mo:mo + 1])
        iw2 = pro.tile([P, 4, D], BF16, tag="iw")
        nc.sync.dma_start(iw2[:], icwT[:])
        ps0b = prop.tile([P, 4, NB], F32, tag="h0p")
        for mo in range(4):
            for ko in range(4):
                nc.tensor.matmul(ps0b[:, mo, :], lhsT=iw2[:, ko, ts(mo, P)],
                                 rhs=avg[:, ko, :], start=ko == 0, stop=ko == 3)
        for mo in range(4):
            nc.scalar.activation(c[:, mo, :], ps0b[:, mo, :], AF.Tanh,
                                 bias=icb[:, mo:mo + 1])

        # W_s = W_w @ img + W_b, streamed over i-slabs of 256
        ww = pro.tile([P, 4, D], BF16, tag="ww")
        nc.sync.dma_start(ww[:], WwT[:])
        for s in range(25):
            w = 256 if s < 24 else BR - 24 * 256
            sl = ds(s * 256, w)
            it = pro2.tile([P, 4, 256], BF16, tag="imgs")
            nc.sync.dma_start(it[:, :, :w], img_eT[:, :, sl])
            psw = prow.tile([P, 4, 256], F32, tag="wsp")
            for mo in range(4):
                for ko in range(4):
                    nc.tensor.matmul(psw[:, mo, :w], lhsT=ww[:, ko, ts(mo, P)],
                                     rhs=it[:, ko, :w], start=ko == 0, stop=ko == 3)
            nc.vector.tensor_tensor(
                out=ws[:, :, sl], in0=psw[:, :, :w],
                in1=wb4[:, :, None].to_broadcast([P, 4, w]), op=OP.add)

    # ---- main loop ----
    with tc.tile_pool(name="work", bufs=2) as work, \
         tc.tile_pool(name="wk1", bufs=1) as wk1, \
         tc.tile_pool(name="attp", bufs=2) as attp, \
         tc.tile_pool(name="pse", bufs=1, space="PSUM") as pse_p, \
         tc.tile_pool(name="psg", bufs=1, space="PSUM") as psg_p, \
         tc.tile_pool(name="psm", bufs=2, space="PSUM") as psm_p, \
         tc.tile_pool(name="pst", bufs=1, space="PSUM") as pst_p:
        for t in range(tsteps):
            hprev = h0[:] if t == 0 else H[:, :, ts(t - 1, NB)]

            # u = U h + U_b (bf16 for the DVE broadcast add)
            psu = psm_p.tile([P, 4, NB], F32, tag="med")
            for mo in range(4):
                for ko in range(4):
                    nc.tensor.matmul(psu[:, mo, :], lhsT=uw[:, ko, ts(mo, P)],
                                     rhs=hprev[:, ko, :], start=ko == 0,
                                     stop=ko == 3)
            u_sb = work.tile([P, 4, NB], BF16, tag="u")
            nc.vector.tensor_tensor(out=u_sb[:], in0=psu[:],
                                    in1=ub4[:, :, None].to_broadcast([P, 4, NB]),
                                    op=OP.add)

            # attention: DVE bcast-add -> ACT tanh -> PE e-reduce, chunked
            pse = pse_p.tile([P, NJ], F32, tag="e")
            r0 = 0
            while r0 < R:
                nr = min(RPC, R - r0)
                i0 = r0 * NB
                att = attp.tile([P, 4, RPC, NB], BF16, tag="att")
                wsv = ws[:, :, ds(i0, nr * NB)].rearrange(
                    "p d (r b) -> p d r b", b=NB)
                nc.vector.tensor_tensor(
                    out=att[:, :, :nr, :], in0=wsv,
                    in1=u_sb[:, :, None, :].to_broadcast([P, 4, nr, NB]),
                    op=OP.add)
                nc.scalar.activation(att[:, :, :nr, :], att[:, :, :nr, :],
                                     AF.Tanh)
                attf = att[:].rearrange("p d r b -> p d (r b)")
                for sc in range(nr * NB // P):
                    j = i0 // P + sc
                    for do in range(4):
                        nc.tensor.matmul(pse[:, j:j + 1],
                                         lhsT=attf[:, do, ts(sc, P)],
                                         rhs=v4t[:, do:do + 1],
                                         start=do == 0, stop=do == 3)
                r0 += nr

            # softmax over r (v_b cancels; |e| is small so no max-subtract)
            alf = alf_all[:, t, :]
            esum = work.tile([P, 1], F32, tag="esum")
            nc.scalar.activation(alf, pse[:], AF.Exp, accum_out=esum[:])
            psS = pst_p.tile([NB, 1], F32, tag="tiny")
            nc.tensor.matmul(psS[:], lhsT=sbp[:], rhs=esum[:], start=True,
                             stop=True)
            sinv = work.tile([NB, 1], F32, tag="sinv")
            nc.vector.reciprocal(sinv[:], psS[:])
            psSb = pst_p.tile([P, 1], F32, tag="tiny")
            nc.tensor.matmul(psSb[:], lhsT=spb[:], rhs=sinv[:], start=True,
                             stop=True)
            nc.vector.tensor_tensor(out=alf, in0=alf,
                                    in1=psSb[:].to_broadcast([P, NJ]),
                                    op=OP.mult)
            # alpha -> bf16 pad -> 32x32 block transpose
            nc.vector.tensor_copy(out=apad[:, :NJ], in_=alf)
            alT = work.tile([P, 64], BF16, tag="alT")
            nc.vector.transpose(alT[:], apad[:])

            # ctx = sum_r alpha f (f stationary per (b, e-chunk))
            psc = psm_p.tile([P, 4, NB], F32, tag="med")
            for b in range(NB):
                for ec in range(4):
                    nc.tensor.matmul(psc[:, ec, b:b + 1],
                                     lhsT=fr0[:, b, ts(ec, P)],
                                     rhs=alT[:, b:b + 1], start=True, stop=False)
                    nc.tensor.matmul(psc[:, ec, b:b + 1],
                                     lhsT=fr1[:, b, ts(ec, P)],
                                     rhs=alT[:, 32 + b:33 + b], start=False,
                                     stop=True)

            # gate = sigmoid(fb_w h + fb_b) = 0.5 tanh((fb_w h + fb_b)/2) + 0.5
            psf = psm_p.tile([P, 4, NB], F32, tag="med")
            for mo in range(4):
                for ko in range(4):
                    nc.tensor.matmul(psf[:, mo, :], lhsT=fbw[:, ko, ts(mo, P)],
                                     rhs=hprev[:, ko, :], start=ko == 0,
                                     stop=ko == 3)
            tg = wk1.tile([P, 4, NB], F32, tag="tg")
            for mo in range(4):
                nc.scalar.activation(tg[:, mo, :], psf[:, mo, :], AF.Tanh,
                                     bias=fbb[:, mo:mo + 1], scale=0.5)
            gate = wk1.tile([P, 4, NB], F32, tag="gate")
            nc.vector.tensor_scalar(out=gate[:], in0=tg[:], scalar1=0.5,
                                    scalar2=0.5, op0=OP.mult, op1=OP.add)
            x2 = work.tile([P, 4, NB], BF16, tag="x2")
            nc.vector.tensor_tensor(out=x2[:], in0=gate[:], in1=psc[:],
                                    op=OP.mult)

            # LSTM gates = W_ih [emb_t, x2] + W_hh h + (b_ih + b_hh)
            psg = psg_p.tile([P, 16, NB], F32, tag="gates")
            et = embT[:, :, ts(t, NB)]
            for go in range(16):
                for ko in range(4):
                    nc.tensor.matmul(psg[:, go, :], lhsT=wih[:, ko, ts(go, P)],
                                     rhs=et[:, ko, :], start=ko == 0, stop=False)
                for ko in range(4):
                    nc.tensor.matmul(psg[:, go, :],
                                     lhsT=wih[:, 4 + ko, ts(go, P)],
                                     rhs=x2[:, ko, :], start=False, stop=False)
                for ko in range(4):
                    nc.tensor.matmul(psg[:, go, :], lhsT=whh[:, ko, ts(go, P)],
                                     rhs=hprev[:, ko, :], start=False,
                                     stop=ko == 3)
            gsb = wk1.tile([P, 16, NB], F32, tag="gsb")
            nc.vector.tensor_tensor(out=gsb[:], in0=psg[:],
                                    in1=b16[:, :, None].to_broadcast([P, 16, NB]),
                                    op=OP.add)
            th = gsb
            nc.scalar.activation(th[:, 0:8], gsb[:, 0:8], AF.Tanh, scale=0.5)
            nc.scalar.activation(th[:, 8:12], gsb[:, 8:12], AF.Tanh)
            nc.scalar.activation(th[:, 12:16], gsb[:, 12:16], AF.Tanh, scale=0.5)
            sif = wk1.tile([P, 8, NB], F32, tag="sif")
            nc.vector.tensor_scalar(out=sif[:], in0=th[:, 0:8], scalar1=0.5,
                                    scalar2=0.5, op0=OP.mult, op1=OP.add)
            # c = sig_f * c + sig_i * tanh(g); h = sig_o * tanh(c)
            nc.vector.tensor_tensor(out=c[:], in0=c[:], in1=sif[:, 4:8],
                                    op=OP.mult)
            tmp = wk1.tile([P, 4, NB], F32, tag="tmp")
            nc.vector.tensor_tensor(out=tmp[:], in0=sif[:, 0:4],
                                    in1=th[:, 8:12], op=OP.mult)
            nc.vector.tensor_tensor(out=c[:], in0=c[:], in1=tmp[:], op=OP.add)
            thc = wk1.tile([P, 4, NB], F32, tag="thc")
            nc.scalar.activation(thc[:], c[:], AF.Tanh)
            sio = wk1.tile([P, 4, NB], F32, tag="sio")
            nc.vector.tensor_scalar(out=sio[:], in0=th[:, 12:16], scalar1=0.5,
                                    scalar2=0.5, op0=OP.mult, op1=OP.add)
            nc.vector.tensor_tensor(out=H[:, :, ts(t, NB)], in0=thc[:],
                                    in1=sio[:], op=OP.mult)

    # alphas out: dest[(r%4)*32+b, t, r//4] = alphas[b, t, 4*(r//4)+(r%4)]
    avw = alphas[:].rearrange("b t (j rl) -> rl b t j", rl=4)
    for rl in range(4):
        nc.sync.dma_start(avw[rl], alf_all[ts(rl, NB)])

    # ---- output projection: preds[bt] = H^T @ out_w^T + out_b ----
    nmc = ngr
    with tc.tile_pool(name="ow", bufs=2) as owp, \
         tc.tile_pool(name="ob", bufs=3) as obp, \
         tc.tile_pool(name="obb", bufs=1) as obbp, \
         tc.tile_pool(name="opp", bufs=4, space="PSUM") as opp, \
         tc.tile_pool(name="opb", bufs=2, space="PSUM") as opbp:
        pv = preds[:].rearrange("b t v -> t b v")
        for vci in range(20):
            ow = owp.tile([P, 4, 500], BF16, tag="ow")
            nc.sync.dma_start(ow[:], owT[:, :, ts(vci, 500)])
            owbt = obbp.tile([1, 500], BF16, tag="owbt")
            nc.sync.dma_start(owbt[:], owb[:, ts(vci, 500)])
            # broadcast out_b chunk to all partitions via K=1 ones matmul
            obc = obbp.tile([P, 500], F32, tag="obc")
            psb = opbp.tile([P, 500], F32, tag="opb")
            nc.tensor.matmul(psb[:], lhsT=onesr[:], rhs=owbt[:], start=True,
                             stop=True)
            nc.vector.tensor_copy(out=obc[:], in_=psb[:])
            for mc in range(nmc):
                mrows = min(P, bt - mc * P)
                vsl = ds(vci * 500, 500)
                pso = opp.tile([P, 500], F32, tag="op")
                for ko in range(4):
                    nc.tensor.matmul(pso[:mrows, :],
                                     lhsT=H[:, ko, ds(mc * P, mrows)],
                                     rhs=ow[:, ko, :],
                                     start=ko == 0, stop=ko == 3)
                ob = obp.tile([P, 500], F32, tag="ob")
                nc.vector.tensor_tensor(out=ob[:mrows, :], in0=pso[:mrows, :],
                                        in1=obc[:mrows, :], op=OP.add)
                for tt in range(mrows // NB):
                    nc.sync.dma_start(pv[mc * 4 + tt, :, vsl],
                                      ob[ts(tt, NB), :])

# ---------------- host side ----------------

def _split4(w):
    # [X] -> [128, X/128] f32 with x = xo*128 + xi
    n = w.shape[0] // P
    return np.ascontiguousarray(w.reshape(n, P).T.astype(np.float32))


def _wT(w):
    # [out, in] weight -> lhsT layout [128(k_in), k_out, out] bf16
    k = w.shape[1]
    return np.ascontiguousarray(
        w.T.reshape(k // P, P, w.shape[0]).transpose(1, 0, 2).astype(bf16))


def prep_shared(U_w, U_b, W_w, W_b, v_w, ih_w, ih_b, ic_w, ic_b,
                fb_w, fb_b, out_w, out_b, emb, W_ih, W_hh, b_ih, b_hh):
    sel = (np.arange(P)[:, None] % NB == np.arange(NB)[None, :]).astype(np.float32)
    return {
        "WwT": _wT(W_w), "UwT": _wT(U_w), "fbwT": _wT(fb_w),
        "ihwT": _wT(ih_w / R), "icwT": _wT(ic_w / R),
        "WihT": _wT(W_ih), "WhhT": _wT(W_hh), "owT": _wT(out_w),
        "owb": np.ascontiguousarray(out_b.reshape(1, V).astype(bf16)),
        "embt": emb.astype(bf16),
        "v4": np.ascontiguousarray(v_w.reshape(4, P).T.astype(bf16)),
        "selbp": sel, "selpb": np.ascontiguousarray(sel.T),
        "Ub4": _split4(U_b), "Wb4": _split4(W_b), "fbb4h": _split4(fb_b / 2),
        "ihb4": _split4(ih_b), "icb4": _split4(ic_b),
        "bih16": _split4(b_ih + b_hh),
    }


def prep_core(img_k, cap_k, tsteps=TT):
    # img_k [NB, R, E] f32, cap_k [NB, T] int
    img_eT = np.ascontiguousarray(
        img_k.reshape(NB, R, 4, P).transpose(3, 2, 1, 0).reshape(P, 4, BR)
        .astype(bf16))
    imgp = np.zeros((NB, 256, E), np.float32)
    imgp[:, :R] = img_k
    a = imgp.reshape(NB, 64, 4, E).transpose(2, 1, 0, 3)  # [rl, jg, b, e]
    f0 = np.ascontiguousarray(a[:, :32].reshape(P, NB, E).astype(bf16))
    f1 = np.ascontiguousarray(a[:, 32:].reshape(P, NB, E).astype(bf16))
    idx = np.zeros(BTP, np.int32)
    idx[:NB * tsteps] = cap_k[:, :tsteps].astype(np.int32).T.reshape(-1)
    cap = np.ascontiguousarray(idx.reshape(7, P).T)
    return {"img_eT": img_eT, "f0": f0, "f1": f1, "cap": cap}


_NC_CACHE = {}


def kernel(img_features, captions, U_w, U_b, W_w, W_b, v_w, v_b,
           ih_w, ih_b, ic_w, ic_b, fb_w, fb_b, out_w, out_b,
           emb, W_ih, W_hh, b_ih, b_hh):
    from concourse.bass_utils import run_bass_kernel_spmd

    img_features = np.asarray(img_features, dtype=np.float32)
    captions = np.asarray(captions)
    shared = prep_shared(
        U_w=np.asarray(U_w, np.float32), U_b=np.asarray(U_b, np.float32),
        W_w=np.asarray(W_w, np.float32), W_b=np.asarray(W_b, np.float32),
        v_w=np.asarray(v_w, np.float32),
        ih_w=np.asarray(ih_w, np.float32), ih_b=np.asarray(ih_b, np.float32),
        ic_w=np.asarray(ic_w, np.float32), ic_b=np.asarray(ic_b, np.float32),
        fb_w=np.asarray(fb_w, np.float32), fb_b=np.asarray(fb_b, np.float32),
        out_w=np.asarray(out_w, np.float32), out_b=np.asarray(out_b, np.float32),
        emb=np.asarray(emb, np.float32),
        W_ih=np.asarray(W_ih, np.float32), W_hh=np.asarray(W_hh, np.float32),
        b_ih=np.asarray(b_ih, np.float32), b_hh=np.asarray(b_hh, np.float32))
    in_maps = []
    for k in range(NCORES):
        m = dict(shared)
        m.update(prep_core(img_features[k * NB:(k + 1) * NB],
                           captions[k * NB:(k + 1) * NB]))
        in_maps.append(m)

    if "nc" not in _NC_CACHE:
        _NC_CACHE["nc"] = build_nc()
    nc = _NC_CACHE["nc"]
    res = run_bass_kernel_spmd(nc, in_maps, core_ids=list(range(NCORES)))
    preds = np.concatenate([r["preds"] for r in res.results], axis=0)
    alphas = np.concatenate([r["alphas"] for r in res.results], axis=0)
    return preds.astype(np.float32), alphas.astype(np.float32)


# revision 23
# speedup vs baseline: 1.2691x; 1.2691x over previous
"""Trainium2 Bass kernel for the attention-LSTM image-captioning decoder
(Show-Attend-Tell style). Full inputs in, full outputs out.

Sharding: data-parallel over batch across 8 NeuronCores (32 rows each),
weights replicated; the sequential 25-step time loop runs locally per core.

v2: matmul-count-optimized. The v1 profile showed PE instruction overhead
dominating (18K LDWEIGHTS+MATMUL pairs at ~110-185ns each). v2 keeps the
e-score reduction in its "att as stationary weights, N=1" form (which lands
e directly in the [p=(r%4)*32+b, j=r//4] layout softmax needs) but moves
everything else to few, wide matmuls:
  - ctx: alpha expanded to a block-diagonal [128, 49, 32] bf16 tile by one
    masked DVE multiply, then 49 accumulating [K=128, M=32, N=512] matmuls
    against f in (b,r)-flattened layout.
  - LSTM/gates: batch-partition orientation [32, 2048] with inputs as
    stationary [128, 32] tiles and weights streamed as N=512 moving
    operands; biases folded in via K=1 ones-row matmuls; h/x2 transposed
    back to d-partitions with PE transposes.
  - sigmoid = 0.5*tanh(x/2)+0.5 keeps everything in the exp/tanh table set.
Output projection: H archive (d-partitions) stationary, out_w^T streamed.
"""

import math
import numpy as np
import ml_dtypes
from contextlib import ExitStack

import concourse.bass as bass
import concourse.mybir as mybir
import concourse.tile as tile
from concourse import bacc
from concourse.bass import ts, ds
from concourse.masks import make_identity

P = 128
B, R, E, D, V, T = 256, 196, 512, 512, 10000, 26
NCORES = 8
NB = B // NCORES          # 32 batch rows per core
TT = T - 1                # 25 time steps
BR = NB * R               # 6272
NJ = BR // P              # 49
RPC = 20                  # r per attention chunk (20*32 = 5 i-subchunks)
G4 = 4 * D                # 2048
F32 = mybir.dt.float32
BF16 = mybir.dt.bfloat16
I32 = mybir.dt.int32
AF = mybir.ActivationFunctionType
OP = mybir.AluOpType
bf16 = ml_dtypes.bfloat16


def build_nc(tsteps=TT):
    nc = bacc.Bacc("TRN2", target_bir_lowering=False, debug=False)
    dp = nc.declare_dram_parameter
    img_eT = dp("img_eT", [P, 4, BR], BF16, isOutput=False)   # [e_in, e_out, i]
    f_br = dp("f_br", [P, NJ, E], BF16, isOutput=False)       # [i%128, i//128, e]
    cap = dp("cap", [P, 7], I32, isOutput=False)              # emb row idx, bt-major
    WwT = dp("WwT", [P, 4, D], BF16, isOutput=False)
    UwT = dp("UwT", [P, 4, D], BF16, isOutput=False)
    fbwT = dp("fbwT", [P, 4, E], BF16, isOutput=False)
    ihwT = dp("ihwT", [P, 4, D], BF16, isOutput=False)        # pre-scaled by 1/R
    icwT = dp("icwT", [P, 4, D], BF16, isOutput=False)
    Wih1T = dp("Wih1T", [P, 4, G4], BF16, isOutput=False)     # W_ih[:, :512].T
    Wih2T = dp("Wih2T", [P, 4, G4], BF16, isOutput=False)     # W_ih[:, 512:].T
    WhhT = dp("WhhT", [P, 4, G4], BF16, isOutput=False)
    owT = dp("owT", [P, 4, V], BF16, isOutput=False)
    owb = dp("owb", [1, V], BF16, isOutput=False)
    embt = dp("embt", [V, D], BF16, isOutput=False)
    v4 = dp("v4", [P, 4], BF16, isOutput=False)
    selbp = dp("selbp", [P, NB], F32, isOutput=False)
    selpb = dp("selpb", [NB, P], F32, isOutput=False)
    maskbf = dp("maskbf", [P, NB], BF16, isOutput=False)
    Ub4 = dp("Ub4", [P, 4], F32, isOutput=False)
    Wb4 = dp("Wb4", [P, 4], F32, isOutput=False)
    fbbrow = dp("fbbrow", [1, E], BF16, isOutput=False)
    ihbrow = dp("ihbrow", [1, D], BF16, isOutput=False)
    icbrow = dp("icbrow", [1, D], BF16, isOutput=False)
    bihrow = dp("bihrow", [1, G4], BF16, isOutput=False)      # b_ih + b_hh
    preds = dp("preds", [NB, tsteps, V], F32, isOutput=True)
    alphas = dp("alphas", [NB, tsteps, R], F32, isOutput=True)

    with tile.TileContext(nc) as tc:
        with ExitStack() as ctx:
            build_body(ctx, tc, tsteps, img_eT, f_br, cap, WwT, UwT, fbwT,
                       ihwT, icwT, Wih1T, Wih2T, WhhT, owT, owb, embt, v4,
                       selbp, selpb, maskbf, Ub4, Wb4, fbbrow, ihbrow, icbrow,
                       bihrow, preds, alphas)
    nc.compile()
    return nc


def build_body(ctx, tc, tsteps, img_eT, f_br, cap, WwT, UwT, fbwT, ihwT,
               icwT, Wih1T, Wih2T, WhhT, owT, owb, embt, v4, selbp, selpb,
               maskbf, Ub4, Wb4, fbbrow, ihbrow, icbrow, bihrow, preds,
               alphas):
    nc = tc.nc
    bt = NB * tsteps
    ngr = math.ceil(bt / P)
    btp = ngr * P
    const = ctx.enter_context(tc.tile_pool(name="const", bufs=1))

    # ---- resident SBUF tiles ----
    fbr = const.tile([P, NJ, E], BF16, tag="fbr")
    nc.sync.dma_start(fbr[:], f_br[:])
    wih1 = const.tile([P, 4, G4], BF16, tag="wih1")
    wih2 = const.tile([P, 4, G4], BF16, tag="wih2")
    whh = const.tile([P, 4, G4], BF16, tag="whh")
    uw = const.tile([P, 4, D], BF16, tag="uw")
    fbw = const.tile([P, 4, E], BF16, tag="fbw")
    nc.sync.dma_start(wih1[:], Wih1T[:])
    nc.sync.dma_start(wih2[:], Wih2T[:])
    nc.sync.dma_start(whh[:], WhhT[:])
    nc.sync.dma_start(uw[:], UwT[:])
    nc.sync.dma_start(fbw[:], fbwT[:])
    v4t = const.tile([P, 4], BF16, tag="v4t")
    nc.sync.dma_start(v4t[:], v4[:])
    sbp = const.tile([P, NB], F32, tag="sbp")
    spb = const.tile([NB, P], F32, tag="spb")
    mbf = const.tile([P, NB], BF16, tag="mbf")
    nc.sync.dma_start(sbp[:], selbp[:])
    nc.sync.dma_start(spb[:], selpb[:])
    nc.sync.dma_start(mbf[:], maskbf[:])
    ub4 = const.tile([P, 4], F32, tag="ub4")
    wb4 = const.tile([P, 4], F32, tag="wb4")
    nc.sync.dma_start(ub4[:], Ub4[:])
    nc.sync.dma_start(wb4[:], Wb4[:])
    fbbr = const.tile([1, E], BF16, tag="fbbr")
    ihbr = const.tile([1, D], BF16, tag="ihbr")
    icbr = const.tile([1, D], BF16, tag="icbr")
    bihr = const.tile([1, G4], BF16, tag="bihr")
    nc.sync.dma_start(fbbr[:], fbbrow[:])
    nc.sync.dma_start(ihbr[:], ihbrow[:])
    nc.sync.dma_start(icbr[:], icbrow[:])
    nc.sync.dma_start(bihr[:], bihrow[:])
    capt = const.tile([P, 7], I32, tag="capt")
    nc.sync.dma_start(capt[:], cap[:])

    ws = const.tile([P, 4, BR], BF16, tag="ws")        # W_s, free order i
    embT = const.tile([P, 4, btp], BF16, tag="embT")   # teacher-forced emb^T
    H = const.tile([P, 4, btp], BF16, tag="H")         # h^T archive; h0 in pad
    h_sb = const.tile([NB, D], BF16, tag="h_sb")       # h state, b-partitions
    c = const.tile([NB, D], F32, tag="c")              # c state, b-partitions
    ident = const.tile([P, P], BF16, tag="ident")
    make_identity(nc, ident[:])
    onesr = const.tile([1, P], BF16, tag="onesr")
    nc.gpsimd.memset(onesr[:], 1.0)
    hT0 = H[:, :, ds(bt, NB)] if btp - bt >= NB else None
    assert hT0 is not None

    # ---- prologue ----
    with tc.tile_pool(name="pro", bufs=1) as pro, \
         tc.tile_pool(name="pro2", bufs=2) as pro2, \
         tc.tile_pool(name="prop", bufs=1, space="PSUM") as prop, \
         tc.tile_pool(name="prow", bufs=2, space="PSUM") as prow, \
         tc.tile_pool(name="proww", bufs=2, space="PSUM") as proww:
        # emb gather + transpose to embT
        for g in range(ngr):
            gt = pro2.tile([P, D], BF16, tag="gather")
            nc.gpsimd.indirect_dma_start(
                out=gt[:], out_offset=None, in_=embt[:],
                in_offset=bass.IndirectOffsetOnAxis(ap=capt[:, g:g + 1], axis=0))
            for dc in range(4):
                pst = prow.tile([P, P], BF16, tag="tpsum")
                nc.tensor.transpose(pst[:], gt[:, ts(dc, P)], ident[:])
                nc.vector.tensor_copy(out=embT[:, dc, ts(g, P)], in_=pst[:])

        # avg (sum over r; 1/R folded into ihwT/icwT): 49 selection matmuls
        psa = prop.tile([NB, E], F32, tag="pp")
        for j in range(NJ):
            nc.tensor.matmul(psa[:], lhsT=mbf[:], rhs=fbr[:, j, :],
                             start=j == 0, stop=j == NJ - 1)
        avgb = pro.tile([NB, E], BF16, tag="avgb")
        nc.vector.tensor_copy(out=avgb[:], in_=psa[:])
        avgT = pro.tile([P, 4, NB], BF16, tag="avgT")
        for ec in range(4):
            pst2 = prop.tile([P, NB], BF16, tag="tp2")
            nc.tensor.transpose(pst2[:], avgb[:, ts(ec, P)],
                                ident[:NB, :NB])
            nc.vector.tensor_copy(out=avgT[:, ec, :], in_=pst2[:])

        # h0 = tanh(avg/R @ ih_w.T + ih_b), c0 likewise (b-partitions)
        iw = pro.tile([P, 4, D], BF16, tag="iw")
        nc.sync.dma_start(iw[:], ihwT[:])
        ps0 = prop.tile([NB, D], F32, tag="pp")
        for eo in range(4):
            nc.tensor.matmul(ps0[:], lhsT=avgT[:, eo, :], rhs=iw[:, eo, :],
                             start=eo == 0, stop=False)
        nc.tensor.matmul(ps0[:], lhsT=onesr[:, :NB], rhs=ihbr[:],
                         start=False, stop=True)
        nc.scalar.activation(h_sb[:], ps0[:], AF.Tanh)
        iw2 = pro.tile([P, 4, D], BF16, tag="iw")
        nc.sync.dma_start(iw2[:], icwT[:])
        ps0b = prop.tile([NB, D], F32, tag="pp")
        for eo in range(4):
            nc.tensor.matmul(ps0b[:], lhsT=avgT[:, eo, :], rhs=iw2[:, eo, :],
                             start=eo == 0, stop=False)
        nc.tensor.matmul(ps0b[:], lhsT=onesr[:, :NB], rhs=icbr[:],
                         start=False, stop=True)
        nc.scalar.activation(c[:], ps0b[:], AF.Tanh)
        # h0^T into the H pad slot
        for ec in range(4):
            pst3 = prop.tile([P, NB], BF16, tag="tp2")
            nc.tensor.transpose(pst3[:], h_sb[:, ts(ec, P)], ident[:NB, :NB])
            nc.vector.tensor_copy(out=hT0[:, ec, :], in_=pst3[:])

        # W_s = W_w @ img + W_b, streamed over i-slabs of 256
        ww = pro.tile([P, 4, D], BF16, tag="ww")
        nc.sync.dma_start(ww[:], WwT[:])
        for s in range(25):
            w = 256 if s < 24 else BR - 24 * 256
            sl = ds(s * 256, w)
            it = pro2.tile([P, 4, 256], BF16, tag="imgs")
            nc.sync.dma_start(it[:, :, :w], img_eT[:, :, sl])
            psw = proww.tile([P, 4, 256], F32, tag="wsp")
            for mo in range(4):
                for ko in range(4):
                    nc.tensor.matmul(psw[:, mo, :w], lhsT=ww[:, ko, ts(mo, P)],
                                     rhs=it[:, ko, :w], start=ko == 0,
                                     stop=ko == 3)
            nc.vector.tensor_tensor(
                out=ws[:, :, sl], in0=psw[:, :, :w],
                in1=wb4[:, :, None].to_broadcast([P, 4, w]), op=OP.add)

    # ---- main loop ----
    with tc.tile_pool(name="work", bufs=2) as work, \
         tc.tile_pool(name="wk1", bufs=1) as wk1, \
         tc.tile_pool(name="attp", bufs=2) as attp, \
         tc.tile_pool(name="pse", bufs=1, space="PSUM") as pse_p, \
         tc.tile_pool(name="psu", bufs=2, space="PSUM") as psu_p, \
         tc.tile_pool(name="psc", bufs=1, space="PSUM") as psc_p, \
         tc.tile_pool(name="psg", bufs=1, space="PSUM") as psg_p, \
         tc.tile_pool(name="pstp", bufs=2, space="PSUM") as pstp_p:
        for t in range(tsteps):
            hT = hT0 if t == 0 else H[:, :, ts(t - 1, NB)]

            # u = U h + U_b  (d-partitions, for the attention broadcast add)
            psu = psu_p.tile([P, 4, NB], F32, tag="sm")
            for mo in range(4):
                for ko in range(4):
                    nc.tensor.matmul(psu[:, mo, :], lhsT=uw[:, ko, ts(mo, P)],
                                     rhs=hT[:, ko, :], start=ko == 0,
                                     stop=ko == 3)
            u_sb = work.tile([P, 4, NB], BF16, tag="u")
            nc.vector.tensor_tensor(out=u_sb[:], in0=psu[:],
                                    in1=ub4[:, :, None].to_broadcast([P, 4, NB]),
                                    op=OP.add)

            # attention: DVE bcast-add -> ACT tanh (in place) -> PE e-reduce
            pse = pse_p.tile([P, NJ], F32, tag="e")
            r0 = 0
            while r0 < R:
                nr = min(RPC, R - r0)
                i0 = r0 * NB
                att = attp.tile([P, 4, RPC, NB], BF16, tag="att")
                wsv = ws[:, :, ds(i0, nr * NB)].rearrange(
                    "p d (r b) -> p d r b", b=NB)
                nc.vector.tensor_tensor(
                    out=att[:, :, :nr, :], in0=wsv,
                    in1=u_sb[:, :, None, :].to_broadcast([P, 4, nr, NB]),
                    op=OP.add)
                nc.scalar.activation(att[:, :, :nr, :], att[:, :, :nr, :],
                                     AF.Tanh)
                attf = att[:].rearrange("p d r b -> p d (r b)")
                for sc in range(nr * NB // P):
                    j = i0 // P + sc
                    for do in range(4):
                        nc.tensor.matmul(pse[:, j:j + 1],
                                         lhsT=attf[:, do, ts(sc, P)],
                                         rhs=v4t[:, do:do + 1],
                                         start=do == 0, stop=do == 3)
                r0 += nr

            # softmax over r (v_b cancels; |e| small so no max-subtract)
            alft = work.tile([P, NJ], F32, tag="alf")
            alf = alft[:]
            esum = work.tile([P, 1], F32, tag="esum")
            nc.scalar.activation(alf, pse[:], AF.Exp, accum_out=esum[:])
            psS = psu_p.tile([NB, 1], F32, tag="sm")
            nc.tensor.matmul(psS[:], lhsT=sbp[:], rhs=esum[:], start=True,
                             stop=True)
            sinv = work.tile([NB, 1], F32, tag="sinv")
            nc.vector.reciprocal(sinv[:], psS[:])
            psSb = psu_p.tile([P, 1], F32, tag="sm")
            nc.tensor.matmul(psSb[:], lhsT=spb[:], rhs=sinv[:], start=True,
                             stop=True)
            nc.vector.tensor_tensor(out=alf, in0=alf,
                                    in1=psSb[:].to_broadcast([P, NJ]),
                                    op=OP.mult)

            # alphas out (per step, partition-group strided writes)
            avw = alphas[:].rearrange("b t (j rl) -> rl b t j", rl=4)
            for rl in range(4):
                nc.sync.dma_start(avw[rl, :, t, :], alft[ts(rl, NB), :])

            # alpha~ block-diagonal [128, 49, 32] via masked multiply
            at = wk1.tile([P, NJ, NB], BF16, tag="at")
            nc.vector.tensor_tensor(
                out=at[:], in0=alf[:, :, None].to_broadcast([P, NJ, NB]),
                in1=mbf[:, None, :].to_broadcast([P, NJ, NB]), op=OP.mult)

            # ctx = sum_r alpha f  (b-partitions out)
            psc = psc_p.tile([NB, E], F32, tag="ctx")
            for j in range(NJ):
                nc.tensor.matmul(psc[:], lhsT=at[:, j, :], rhs=fbr[:, j, :],
                                 start=j == 0, stop=j == NJ - 1)

            # gate = sigmoid(fb_w h + fb_b); x2 = gate * ctx (b-partitions)
            psf = psu_p.tile([NB, E], F32, tag="sm")
            for ko in range(4):
                nc.tensor.matmul(psf[:], lhsT=hT[:, ko, :], rhs=fbw[:, ko, :],
                                 start=ko == 0, stop=False)
            nc.tensor.matmul(psf[:], lhsT=onesr[:, :NB], rhs=fbbr[:],
                             start=False, stop=True)
            tg = wk1.tile([NB, E], F32, tag="tg")
            nc.scalar.activation(tg[:], psf[:], AF.Tanh, scale=0.5)
            nc.vector.tensor_scalar(out=tg[:], in0=tg[:], scalar1=0.5,
                                    scalar2=0.5, op0=OP.mult, op1=OP.add)
            x2 = wk1.tile([NB, E], BF16, tag="x2")
            nc.vector.tensor_tensor(out=x2[:], in0=tg[:], in1=psc[:],
                                    op=OP.mult)
            # x2^T (d-partitions) for the gates matmul
            x2T = work.tile([P, 4, NB], BF16, tag="x2T")
            for ec in range(4):
                pst = pstp_p.tile([P, NB], BF16, tag="tp")
                nc.tensor.transpose(pst[:], x2[:, ts(ec, P)], ident[:NB, :NB])
                nc.vector.tensor_copy(out=x2T[:, ec, :], in_=pst[:])

            # gates = W_ih [emb_t, x2] + W_hh h + b  (b-partitions, 2 halves)
            th = wk1.tile([NB, G4], F32, tag="th")
            et = embT[:, :, ts(t, NB)]
            for gh in range(2):
                psg = psg_p.tile([NB, 1024], F32, tag="g")
                for nch in range(2):
                    o = psg[:, ts(nch, 512)]
                    gsl = ds(gh * 1024 + nch * 512, 512)
                    for ko in range(4):
                        nc.tensor.matmul(o, lhsT=et[:, ko, :],
                                         rhs=wih1[:, ko, gsl],
                                         start=ko == 0, stop=False)
                    for ko in range(4):
                        nc.tensor.matmul(o, lhsT=x2T[:, ko, :],
                                         rhs=wih2[:, ko, gsl],
                                         start=False, stop=False)
                    for ko in range(4):
                        nc.tensor.matmul(o, lhsT=hT[:, ko, :],
                                         rhs=whh[:, ko, gsl],
                                         start=False, stop=False)
                    nc.tensor.matmul(o, lhsT=onesr[:, :NB], rhs=bihr[:, gsl],
                                     start=False, stop=True)
                if gh == 0:
                    # i, f gates: sigmoid via tanh(x/2)
                    nc.scalar.activation(th[:, 0:1024], psg[:], AF.Tanh,
                                         scale=0.5)
                else:
                    nc.scalar.activation(th[:, 1024:1536], psg[:, 0:512],
                                         AF.Tanh)
                    nc.scalar.activation(th[:, 1536:2048], psg[:, 512:1024],
                                         AF.Tanh, scale=0.5)

            # c = sig_f * c + sig_i * tanh(g); h = sig_o * tanh(c)
            # sigmoids in place over the tanh outputs: sig = 0.5*th + 0.5
            nc.vector.tensor_scalar(out=th[:, 0:1024], in0=th[:, 0:1024],
                                    scalar1=0.5, scalar2=0.5, op0=OP.mult,
                                    op1=OP.add)
            nc.vector.tensor_scalar(out=th[:, 1536:2048], in0=th[:, 1536:2048],
                                    scalar1=0.5, scalar2=0.5, op0=OP.mult,
                                    op1=OP.add)
            nc.vector.tensor_tensor(out=c[:], in0=c[:], in1=th[:, D:1024],
                                    op=OP.mult)
            tmp = wk1.tile([NB, D], F32, tag="tmp")
            nc.vector.tensor_tensor(out=tmp[:], in0=th[:, :D],
                                    in1=th[:, 1024:1536], op=OP.mult)
            nc.vector.tensor_tensor(out=c[:], in0=c[:], in1=tmp[:], op=OP.add)
            thc = wk1.tile([NB, D], F32, tag="tg")
            nc.scalar.activation(thc[:], c[:], AF.Tanh)
            nc.vector.tensor_tensor(out=h_sb[:], in0=thc[:],
                                    in1=th[:, 1536:2048], op=OP.mult)
            # h^T into the archive (d-partitions)
            for ec in range(4):
                pst = pstp_p.tile([P, NB], BF16, tag="tp")
                nc.tensor.transpose(pst[:], h_sb[:, ts(ec, P)],
                                    ident[:NB, :NB])
                nc.vector.tensor_copy(out=H[:, ec, ts(t, NB)], in_=pst[:])

    # ---- output projection: preds[bt] = H^T @ out_w^T + out_b ----
    nmc = ngr
    with tc.tile_pool(name="ow", bufs=2) as owp, \
         tc.tile_pool(name="ob", bufs=3) as obp, \
         tc.tile_pool(name="obb", bufs=1) as obbp, \
         tc.tile_pool(name="opp", bufs=4, space="PSUM") as opp, \
         tc.tile_pool(name="opb", bufs=2, space="PSUM") as opbp:
        pv = preds[:].rearrange("b t v -> t b v")
        for vci in range(20):
            ow = owp.tile([P, 4, 500], BF16, tag="ow")
            nc.sync.dma_start(ow[:], owT[:, :, ts(vci, 500)])
            owbt = obbp.tile([1, 500], BF16, tag="owbt")
            nc.sync.dma_start(owbt[:], owb[:, ts(vci, 500)])
            # broadcast out_b chunk to all partitions via K=1 ones matmul
            obc = obbp.tile([P, 500], F32, tag="obc")
            psb = opbp.tile([P, 500], F32, tag="opb")
            nc.tensor.matmul(psb[:], lhsT=onesr[:], rhs=owbt[:], start=True,
                             stop=True)
            nc.vector.tensor_copy(out=obc[:], in_=psb[:])
            for mc in range(nmc):
                mrows = min(P, bt - mc * P)
                vsl = ds(vci * 500, 500)
                pso = opp.tile([P, 500], F32, tag="op")
                for ko in range(4):
                    nc.tensor.matmul(pso[:mrows, :],
                                     lhsT=H[:, ko, ds(mc * P, mrows)],
                                     rhs=ow[:, ko, :],
                                     start=ko == 0, stop=ko == 3)
                ob = obp.tile([P, 500], F32, tag="ob")
                nc.vector.tensor_tensor(out=ob[:mrows, :], in0=pso[:mrows, :],
                                        in1=obc[:mrows, :], op=OP.add)
                for tt in range(mrows // NB):
                    nc.sync.dma_start(pv[mc * 4 + tt, :, vsl],
                                      ob[ts(tt, NB), :])


# ---------------- host side ----------------

def _split4(w):
    # [X] -> [128, X/128] f32 with x = xo*128 + xi
    n = w.shape[0] // P
    return np.ascontiguousarray(w.reshape(n, P).T.astype(np.float32))


def _wT(w):
    # [out, in] weight -> lhsT/rhs layout [128(k_in), k_out, out] bf16
    k = w.shape[1]
    return np.ascontiguousarray(
        w.T.reshape(k // P, P, w.shape[0]).transpose(1, 0, 2).astype(bf16))


def prep_shared(U_w, U_b, W_w, W_b, v_w, ih_w, ih_b, ic_w, ic_b,
                fb_w, fb_b, out_w, out_b, emb, W_ih, W_hh, b_ih, b_hh):
    sel = (np.arange(P)[:, None] % NB == np.arange(NB)[None, :])
    return {
        "WwT": _wT(W_w), "UwT": _wT(U_w), "fbwT": _wT(fb_w),
        "ihwT": _wT(ih_w / R), "icwT": _wT(ic_w / R),
        "Wih1T": _wT(W_ih[:, :D]), "Wih2T": _wT(W_ih[:, D:]),
        "WhhT": _wT(W_hh), "owT": _wT(out_w),
        "owb": np.ascontiguousarray(out_b.reshape(1, V).astype(bf16)),
        "embt": emb.astype(bf16),
        "v4": np.ascontiguousarray(v_w.reshape(4, P).T.astype(bf16)),
        "selbp": np.ascontiguousarray(sel.astype(np.float32)),
        "selpb": np.ascontiguousarray(sel.T.astype(np.float32)),
        "maskbf": np.ascontiguousarray(sel.astype(bf16)),
        "Ub4": _split4(U_b), "Wb4": _split4(W_b),
        "fbbrow": np.ascontiguousarray(fb_b.reshape(1, E).astype(bf16)),
        "ihbrow": np.ascontiguousarray(ih_b.reshape(1, D).astype(bf16)),
        "icbrow": np.ascontiguousarray(ic_b.reshape(1, D).astype(bf16)),
        "bihrow": np.ascontiguousarray(
            (b_ih + b_hh).reshape(1, G4).astype(bf16)),
    }


def prep_core(img_k, cap_k, tsteps=TT):
    # img_k [NB, R, E] f32, cap_k [NB, T] int
    img_eT = np.ascontiguousarray(
        img_k.reshape(NB, R, 4, P).transpose(3, 2, 1, 0).reshape(P, 4, BR)
        .astype(bf16))
    arr = img_k.transpose(1, 0, 2).reshape(BR, E)       # i = r*32 + b
    f_br = np.ascontiguousarray(
        arr.reshape(NJ, P, E).transpose(1, 0, 2).astype(bf16))
    btp = math.ceil(NB * tsteps / P) * P
    idx = np.zeros(btp, np.int32)
    idx[:NB * tsteps] = cap_k[:, :tsteps].astype(np.int32).T.reshape(-1)
    cap = np.ascontiguousarray(idx.reshape(btp // P, P).T)
    if cap.shape[1] < 7:
        cap = np.pad(cap, ((0, 0), (0, 7 - cap.shape[1])))
    return {"img_eT": img_eT, "f_br": f_br, "cap": cap}


_NC_CACHE = {}


def kernel(img_features, captions, U_w, U_b, W_w, W_b, v_w, v_b,
           ih_w, ih_b, ic_w, ic_b, fb_w, fb_b, out_w, out_b,
           emb, W_ih, W_hh, b_ih, b_hh):
    from concourse.bass_utils import run_bass_kernel_spmd

    img_features = np.asarray(img_features, dtype=np.float32)
    captions = np.asarray(captions)
    shared = prep_shared(
        U_w=np.asarray(U_w, np.float32), U_b=np.asarray(U_b, np.float32),
        W_w=np.asarray(W_w, np.float32), W_b=np.asarray(W_b, np.float32),
        v_w=np.asarray(v_w, np.float32),
        ih_w=np.asarray(ih_w, np.float32), ih_b=np.asarray(ih_b, np.float32),
        ic_w=np.asarray(ic_w, np.float32), ic_b=np.asarray(ic_b, np.float32),
        fb_w=np.asarray(fb_w, np.float32), fb_b=np.asarray(fb_b, np.float32),
        out_w=np.asarray(out_w, np.float32), out_b=np.asarray(out_b, np.float32),
        emb=np.asarray(emb, np.float32),
        W_ih=np.asarray(W_ih, np.float32), W_hh=np.asarray(W_hh, np.float32),
        b_ih=np.asarray(b_ih, np.float32), b_hh=np.asarray(b_hh, np.float32))
    in_maps = []
    for k in range(NCORES):
        m = dict(shared)
        m.update(prep_core(img_features[k * NB:(k + 1) * NB],
                           captions[k * NB:(k + 1) * NB]))
        in_maps.append(m)

    if "nc" not in _NC_CACHE:
        _NC_CACHE["nc"] = build_nc()
    nc = _NC_CACHE["nc"]
    res = run_bass_kernel_spmd(nc, in_maps, core_ids=list(range(NCORES)))
    preds = np.concatenate([r["preds"] for r in res.results], axis=0)
    alphas = np.concatenate([r["alphas"] for r in res.results], axis=0)
    return preds.astype(np.float32), alphas.astype(np.float32)
